# revision 38
# baseline (speedup 1.0000x reference)
"""FAVOR+ (Performer) multi-head causal attention — Trainium2 Bass kernel.

Sharding: 8 cores = 4 batches x 2 head-groups (4 heads each); no collectives
(host sums the two head-group partials of w_o per batch).

Math note: the softmax-kernel stabilizers and +eps only rescale qp/kp per
(l,h) [or globally], which cancels in num/den up to ~3e-4 relative (verified
numerically). Additionally exp(dd - diag_k) = exp(dd) * g with
g = exp(-0.5|k|^2) a per-position scalar, so g is folded into the v-aug
tensor (per-partition scalar multiply) instead of biasing the kp features:
kp/kplc are plain exp(dd), and v_aug rows (incl. the fused ones columns that
produce s_k/den) are scaled by g.

Precision: f32r (full-rate fp32 streaming) for projections/features/state;
bf16 for the moving operands of the scan state-update matmuls (N=66 would
run at 1/4 rate in f32r), the masked A^T blocks, attnT raw/final and w_o.
Validated ~3e-3 rel err vs reference (tolerance 2e-2).

Schedule notes: all PSUM pools are allocated once up front (no phase
barriers); DMAs are issued x-interleaved-with-weights so the first
projection matmul starts ~3us in; head h+1's feature maps (Act-bound)
overlap head h's scan (DVE/PE-bound) via bufs=2 pools; the v_aug production
(DVE-bound) is emitted between head-0 feature maps and head-0 scan so it
fills the DVE while Act runs Exps.
"""
import numpy as np

B, L, DIM, H, DK, M = 4, 2048, 512, 8, 64, 256
HPC = 4            # heads per core
CW = 256           # scan chunk width
NC2 = L // 128     # 16
NCC = L // CW      # 8
LT = 512
NLT = L // LT

_COMPILED = None


def _build():
    import concourse.bacc as bacc
    import concourse.mybir as mybir
    from concourse.tile import TileContext

    f32 = mybir.dt.float32
    f32r = mybir.dt.float32r
    bf16 = mybir.dt.bfloat16
    EXP = mybir.ActivationFunctionType.Exp
    IDN = mybir.ActivationFunctionType.Identity

    nc = bacc.Bacc("TRN2", target_bir_lowering=False, debug=False,
                   enable_asserts=False, num_devices=8)

    def din(name, shape, dt=f32r):
        return nc.dram_tensor(name, shape, dt, kind="ExternalInput").ap()

    xq = din("xq", [512, L], bf16)
    xk = din("xk", [512, L], bf16)
    xv = din("xv", [512, L], bf16)
    wq = din("wq", [512, 256], bf16)
    wk = din("wk", [512, 256], bf16)
    wv = din("wv", [513, 264], bf16)  # [Wv_slice.T ; bv] with ones cols
    bq2 = din("bq2", [128, 2], f32)
    bk2 = din("bk2", [128, 2], f32)
    projT = din("projT", [128, 256])  # proj.T stacked twice
    bd = din("bd", [128, 8])
    msk = din("msk", [128, 512], f32)
    wo = din("wo", [256, 512], bf16)
    ones = din("ones", [1, 128])
    ones_bf = din("ones_bf", [1, 128], bf16)
    outT = nc.dram_tensor("outT", [512, L], bf16, kind="ExternalOutput").ap()

    with TileContext(nc) as tc:
        with (
            tc.tile_pool(name="const", bufs=1) as cpool,
            tc.tile_pool(name="persist", bufs=1) as ppool,
            tc.tile_pool(name="psQK", bufs=2, space="PSUM") as psQK_pool,
            tc.tile_pool(name="psA", bufs=2, space="PSUM") as psA_pool,
            tc.tile_pool(name="psND", bufs=2, space="PSUM") as psND_pool,
            tc.tile_pool(name="psS", bufs=1, space="PSUM") as psS_pool,
            tc.tile_pool(name="psB", bufs=1, space="PSUM") as psB_pool,
        ):
            def ldconst(name, shape, src, dt=f32r):
                t = cpool.tile(shape, dt, name=name, tag=name)
                nc.sync.dma_start(t[:, :], src)
                return t

            # x pools entered early so their SBUF ranges are independent;
            # xqk (innermost) released right after the q/k projections,
            # then sqx after ksc, then xv after v_aug (LIFO).
            xvin = tc.tile_pool(name="xvin", bufs=1)
            xvpool = xvin.__enter__()
            sqx = tc.tile_pool(name="sqx", bufs=1)
            sqpool = sqx.__enter__()
            xqkin = tc.tile_pool(name="xqkin", bufs=1)
            xqkpool = xqkin.__enter__()

            # ---- DMA issue order: x interleaved with weights ----
            c_wq = [ldconst(f"wq{i}", [128, 256], wq[128 * i:128 * (i + 1), :],
                            bf16) for i in range(4)]
            t_x = {}
            for i in range(4):
                xt = xqkpool.tile([128, L], bf16, name=f"xq{i}", tag=f"xq{i}")
                nc.sync.dma_start(xt[:, :], xq[128 * i:128 * (i + 1), :])
                t_x[("q", i)] = xt
            c_wk = [ldconst(f"wk{i}", [128, 256], wk[128 * i:128 * (i + 1), :],
                            bf16) for i in range(4)]
            for i in range(4):
                xt = xqkpool.tile([128, L], bf16, name=f"xk{i}", tag=f"xk{i}")
                nc.sync.dma_start(xt[:, :], xk[128 * i:128 * (i + 1), :])
                t_x[("k", i)] = xt
            c_projT = ldconst("projT", [128, 256], projT)
            c_bd = ldconst("bd", [128, 8], bd)
            c_wv = [ldconst(f"wv{i}", [128, 264], wv[128 * i:128 * (i + 1), :],
                            bf16) for i in range(4)]
            c_wvb = ldconst("wvb", [1, 264], wv[512:513, :], bf16)
            for i in range(4):
                xt = xvpool.tile([128, L], bf16, name=f"xv{i}", tag=f"xv{i}")
                nc.sync.dma_start(xt[:, :], xv[128 * i:128 * (i + 1), :])
                t_x[("v", i)] = xt
            c_msk = ldconst("msk", [128, 512], msk, f32)
            c_bq = ldconst("bq", [128, 2], bq2, f32)
            c_bk = ldconst("bk", [128, 2], bk2, f32)
            c_ones = ldconst("ones", [1, 128], ones)
            c_ones_bf = ldconst("ones_bf", [1, 128], ones_bf, bf16)
            c_wo = [ldconst(f"wo{i}", [64, 512], wo[64 * i:64 * (i + 1), :],
                            bf16) for i in range(4)]
            c_zero = cpool.tile([128, 132], f32, tag="zero")
            nc.gpsimd.memset(c_zero[:, :], 0.0)

            # persistent activations
            t_qT = [ppool.tile([128, L], f32r, name=f"qT{i}", tag=f"qT{i}")
                    for i in range(2)]
            t_kT = [ppool.tile([128, L], f32r, name=f"kT{i}", tag=f"kT{i}")
                    for i in range(2)]
            t_v = ppool.tile([128, NC2 * 264], bf16, tag="vall")
            t_g = ppool.tile([128, NC2 * 4], f32, tag="gall")

            # ---- P1b: q/k projections + kT^2 (fine-grained) ----
            t_sq = [sqpool.tile([128, L], f32r, name=f"sq{i}", tag=f"sq{i}")
                    for i in range(2)]
            for half in range(2):
                for lt in range(NLT):
                    ls = slice(lt * LT, (lt + 1) * LT)
                    for (wgt, nm, dst, bias) in ((c_wq, "q", t_qT, c_bq),
                                                 (c_wk, "k", t_kT, c_bk)):
                        ps = psQK_pool.tile([128, LT], f32, tag="psQK")
                        for kt in range(4):
                            nc.tensor.matmul(
                                ps[:, :],
                                wgt[kt][:, 128 * half:128 * (half + 1)],
                                t_x[(nm, kt)][:, ls],
                                start=(kt == 0), stop=(kt == 3))
                        nc.scalar.activation(
                            dst[half][:, ls], ps[:, :], IDN,
                            bias=bias[:, half:half + 1])
                    nc.vector.tensor_mul(t_sq[half][:, ls],
                                         t_kT[half][:, ls],
                                         t_kT[half][:, ls])
            xqkin.__exit__(None, None, None)

            # ksc + g = exp(-0.5 |k_h(l)|^2), [128 l, 4 h] per 128-chunk
            for ch in range(NC2):
                cs = slice(ch * 128, (ch + 1) * 128)
                ps = psQK_pool.tile([128, LT], f32, tag="psQK")
                for half in range(2):
                    nc.tensor.matmul(ps[:, 0:4], t_sq[half][:, cs],
                                     c_bd[:, 4 * half:4 * (half + 1)],
                                     start=(half == 0), stop=(half == 1))
                nc.scalar.activation(t_g[:, 4 * ch:4 * (ch + 1)],
                                     ps[:, 0:4], EXP)

            # ---- Phase 2+3: per head (v_aug emitted after head-0 maps) ----
            actx = tc.tile_pool(name="attn", bufs=1)
            apool = actx.__enter__()
            hctx = (tc.tile_pool(name="headbuf", bufs=2),
                    tc.tile_pool(name="work", bufs=3))
            hpool, wpool = [c.__enter__() for c in hctx]
            t_attnT = [apool.tile([64, L], bf16, name=f"attnT{i}",
                                  tag=f"attnT{i}") for i in range(4)]
            octx = tc.tile_pool(name="outp", bufs=3)
            opool = octx.__enter__()
            def alloc_head(h):
                c = {}
                c['qp'] = [hpool.tile([128, L], f32r, name=f"qp{h}_{i}",
                                      tag=f"qp{i}") for i in range(2)]
                c['kp'] = [hpool.tile([128, L], f32r, name=f"kp{h}_{i}",
                                      tag=f"kp{i}") for i in range(2)]
                c['kplc'] = hpool.tile([128, NC2 * 256], bf16,
                                       name=f"kplc{h}", tag="kplc")
                c['Sb'] = [hpool.tile([128, 132], f32r, name=f"S{h}_{i}",
                                      tag=f"S{i}") for i in range(2)]
                c['rcp'] = hpool.tile([1, L], bf16, name=f"rcp{h}",
                                      tag="rcp")
                c['raw'] = hpool.tile([64, L], bf16, name=f"raw{h}",
                                      tag="raw")
                return c

            def partA_units(h, c):
                hh = h // 2
                hr = slice(64 * (h % 2), 64 * (h % 2) + 64)
                pr = hr
                for lt in range(NLT):
                    ls = slice(lt * LT, (lt + 1) * LT)
                    for half in range(2):
                        mh = slice(128 * half, 128 * (half + 1))
                        ps = psQK_pool.tile([128, LT], f32, tag="psQK")
                        nc.tensor.matmul(ps[:, :], c_projT[pr, mh],
                                         t_qT[hh][hr, ls],
                                         start=True, stop=True)
                        nc.scalar.activation(c['qp'][half][:, ls],
                                             ps[:, :], EXP)
                        yield
                        ps2 = psQK_pool.tile([128, LT], f32, tag="psQK")
                        nc.tensor.matmul(ps2[:, :], c_projT[pr, mh],
                                         t_kT[hh][hr, ls],
                                         start=True, stop=True)
                        nc.scalar.activation(c['kp'][half][:, ls],
                                             ps2[:, :], EXP)
                        yield
                for j in range(NCC):  # kplc, two 128-chunks per psum tile
                    cs0 = slice(j * 256, j * 256 + 128)
                    cs1 = slice(j * 256 + 128, (j + 1) * 256)
                    ps = psQK_pool.tile([128, 512], f32, tag="psQK")
                    nc.tensor.matmul(ps[:, 0:256], t_kT[hh][hr, cs0],
                                     c_projT[pr, :], start=True, stop=True)
                    nc.tensor.matmul(ps[:, 256:512], t_kT[hh][hr, cs1],
                                     c_projT[pr, :], start=True, stop=True)
                    nc.scalar.activation(
                        c['kplc'][:, 512 * j:512 * (j + 1)], ps[:, :], EXP)
                    yield

            def emit_partA(h, c):
                for _ in partA_units(h, c):
                    pass

            def emit_scan_chunk(h, c, cc):
                qs = slice(cc * CW, (cc + 1) * CW)
                ts0 = slice(cc * CW, cc * CW + 128)
                ts1 = slice(cc * CW + 128, (cc + 1) * CW)
                t_qp, t_kp, t_kplc = c['qp'], c['kp'], c['kplc']
                t_S = c['Sb'][cc % 2]
                t_Sn = c['Sb'][(cc + 1) % 2]
                psA = psA_pool.tile([128, 512], f32, tag="psA")
                nc.tensor.matmul(psA[:, 0:256], t_kp[0][:, ts0],
                                 t_qp[0][:, qs], start=True, stop=False)
                nc.tensor.matmul(psA[:, 0:256], t_kp[1][:, ts0],
                                 t_qp[1][:, qs], start=False, stop=True)
                nc.tensor.matmul(psA[:, 256:512], t_kp[0][:, ts1],
                                 t_qp[0][:, qs], start=True, stop=False)
                nc.tensor.matmul(psA[:, 256:512], t_kp[1][:, ts1],
                                 t_qp[1][:, qs], start=False, stop=True)
                atm = wpool.tile([128, 512], bf16, tag="atm")
                nc.vector.tensor_mul(atm[:, :], psA[:, :], c_msk[:, :])
                c128 = cc * 2
                va0 = t_v[:, c128 * 264 + h * 66:c128 * 264 + h * 66 + 66]
                va1 = t_v[:, (c128 + 1) * 264 + h * 66:
                          (c128 + 1) * 264 + h * 66 + 66]
                if cc < NCC - 1:
                    psS = psS_pool.tile([128, 132], f32, tag="psS")
                    nc.tensor.matmul(
                        psS[:, 0:66],
                        t_kplc[:, c128 * 256:c128 * 256 + 128],
                        va0, start=True, stop=False)
                    nc.tensor.matmul(
                        psS[:, 0:66],
                        t_kplc[:, (c128 + 1) * 256:(c128 + 1) * 256 + 128],
                        va1, start=False, stop=True)
                    nc.tensor.matmul(
                        psS[:, 66:132],
                        t_kplc[:, c128 * 256 + 128:c128 * 256 + 256],
                        va0, start=True, stop=False)
                    nc.tensor.matmul(
                        psS[:, 66:132],
                        t_kplc[:, (c128 + 1) * 256 + 128:(c128 + 2) * 256],
                        va1, start=False, stop=True)
                    with nc.allow_low_precision(reason="f32r state accumulate (TF32-rounding ~1e-3, validated vs reference)"):
                        nc.vector.tensor_add(t_Sn[:, :], t_S[:, :],
                                             psS[:, :])
                if cc % 2 == 0:
                    c['nd2'] = psND_pool.tile([66, 2 * CW], f32,
                                              name=f"nd2_{h}_{cc}",
                                              tag="psNDt")
                nd2 = c['nd2']
                nd = nd2[:, (cc % 2) * CW:(cc % 2 + 1) * CW]
                nc.tensor.matmul(nd[:, :], va0, atm[:, 0:256],
                                 start=True, stop=False)
                nc.tensor.matmul(nd[:, :], va1, atm[:, 256:512],
                                 start=False, stop=(cc == 0))
                if cc > 0:
                    nc.tensor.matmul(nd[:, :], t_S[:, 0:66],
                                     t_qp[0][:, qs],
                                     start=False, stop=False)
                    nc.tensor.matmul(nd[:, :], t_S[:, 66:132],
                                     t_qp[1][:, qs],
                                     start=False, stop=True)
                if cc % 2 == 1:
                    ds = slice((cc - 1) * CW, (cc + 1) * CW)
                    with nc.allow_low_precision(reason="f32r reciprocal for matmul broadcast (validated vs reference)"):
                        nc.vector.reciprocal(c['rcp'][0:1, ds],
                                             nd2[64:65, :])
                    if cc % 4 == 1:
                        nc.scalar.copy(c['raw'][:, ds], nd2[0:64, :])
                    else:
                        nc.vector.tensor_copy(c['raw'][:, ds], nd2[0:64, :])

            def emit_div_lt(h, c, lt):
                ls = slice(lt * LT, (lt + 1) * LT)
                psB = psB_pool.tile([64, LT], f32, tag="psB")
                nc.tensor.matmul(psB[:, :], c_ones_bf[0:1, 0:64],
                                 c['rcp'][0:1, ls], start=True, stop=True)
                nc.vector.tensor_mul(t_attnT[h][:, ls], c['raw'][:, ls],
                                     psB[:, :])

            def emit_p4_lt(lt):
                ls = slice(lt * LT, (lt + 1) * LT)
                for osub in range(4):
                    os_ = slice(128 * osub, 128 * (osub + 1))
                    ps = psA_pool.tile([128, 512], f32, tag="psA")
                    for hx in range(4):
                        nc.tensor.matmul(ps[:, 0:LT], c_wo[hx][:, os_],
                                         t_attnT[hx][:, ls],
                                         start=(hx == 0), stop=(hx == 3))
                    t_o = opool.tile([128, LT], bf16, tag="outT")
                    nc.scalar.copy(t_o[:, :], ps[:, 0:LT])
                    nc.sync.dma_start(outT[os_, ls], t_o[:, :])

            # Emission interleave: scan(h)'s S-chain bubbles on the in-order
            # PE/Act queues are filled with head h+1's feature-map groups;
            # divisions are emitted at the odd chunk where their rcp lands,
            # and P4 lt-groups stream into scan(3) right after div(3, lt).
            ctxs = [None] * HPC
            ctxs[0] = alloc_head(0)
            emit_partA(0, ctxs[0])
            # v_aug = ((Wv x + bv) | ones) * g -> bf16 per-head slots
            for ch in range(NC2):
                cs = slice(ch * 128, (ch + 1) * 128)
                ps = psA_pool.tile([128, 512], f32, tag="psA")
                for kt in range(4):
                    nc.tensor.matmul(ps[:, 0:264],
                                     t_x[("v", kt)][:, cs],
                                     c_wv[kt][:, :],
                                     start=(kt == 0), stop=False)
                nc.tensor.matmul(ps[:, 0:264], c_ones_bf[0:1, 0:128],
                                 c_wvb[:, :], start=False, stop=True)
                for hv in range(HPC):
                    nc.vector.tensor_scalar_mul(
                        t_v[:, ch * 264 + hv * 66:
                            ch * 264 + (hv + 1) * 66],
                        ps[:, hv * 66:(hv + 1) * 66],
                        t_g[:, 4 * ch + hv:4 * ch + hv + 1])
            for h in range(HPC):
                c0 = ctxs[h]
                if h < HPC - 1:
                    ctxs[h + 1] = alloc_head(h + 1)
                    filler = partA_units(h + 1, ctxs[h + 1])
                else:
                    filler = iter(())
                nc.gpsimd.tensor_copy(c0['Sb'][0][:, :], c_zero[:, :])
                for cc in range(NCC):
                    emit_scan_chunk(h, c0, cc)
                    if cc % 2 == 1 and cc >= 3:
                        lt = (cc - 3) // 2
                        emit_div_lt(h, c0, lt)
                        if h == HPC - 1:
                            emit_p4_lt(lt)
                    for _ in range(4):
                        next(filler, None)
                for _ in filler:
                    pass
                emit_div_lt(h, c0, NLT - 1)
                if h == HPC - 1:
                    emit_p4_lt(NLT - 1)

            octx.__exit__(None, None, None)
            for c in reversed(hctx):
                c.__exit__(None, None, None)
            actx.__exit__(None, None, None)
            sqx.__exit__(None, None, None)
            xvin.__exit__(None, None, None)

    nc.compile()
    return nc


def _prep_inputs(query, key, value, Wq, bq, Wk, bk, Wv, bv, Wo, bo, proj):
    import ml_dtypes
    s = float(DK) ** -0.25
    tri = (np.arange(128)[:, None] <= np.arange(128)[None, :]).astype(np.float32)
    on = np.ones((128, 128), np.float32)
    zr = np.zeros((128, 128), np.float32)
    msk = np.concatenate([tri, on, zr, tri], axis=1)
    bd = np.zeros((128, 8), np.float32)
    for half in range(2):
        for r in range(128):
            bd[r, 4 * half + (2 * half + r // 64)] = -0.5
    pT = np.ascontiguousarray(proj.T)
    common = {"projT": np.concatenate([pT, pT]), "bd": bd, "msk": msk,
              "ones": np.ones((1, 128), np.float32),
              "ones_bf": np.ones((1, 128), ml_dtypes.bfloat16)}
    in_maps = []
    for b in range(B):
        for hg in range(2):
            sl = slice(hg * 256, (hg + 1) * 256)
            Wqs, Wks, Wvs = Wq[sl] * s, Wk[sl] * s, Wv[sl]
            bqs, bks, bvs = bq[sl] * s, bk[sl] * s, bv[sl]
            # wv_aug [513, 264]: per head h cols 66h:66h+64 = Wv.T head cols,
            # cols 66h+64:66h+66 zero; bias row: bv at head cols, 1.0 at ones
            wv_aug = np.zeros((513, 264), np.float32)
            for h in range(HPC):
                wv_aug[0:512, 66 * h:66 * h + 64] = Wvs.T[:, 64 * h:64 * (h + 1)]
                wv_aug[512, 66 * h:66 * h + 64] = bvs[64 * h:64 * (h + 1)]
                wv_aug[512, 66 * h + 64:66 * h + 66] = 1.0
            m = dict(common)
            m["xq"] = np.ascontiguousarray(query[b].T)
            m["xk"] = np.ascontiguousarray(key[b].T)
            m["xv"] = np.ascontiguousarray(value[b].T)
            m["wq"] = np.ascontiguousarray(Wqs.T)
            m["wk"] = np.ascontiguousarray(Wks.T)
            m["wv"] = wv_aug
            m["bq2"] = np.stack([bqs[:128], bqs[128:]], axis=1)
            m["bk2"] = np.stack([bks[:128], bks[128:]], axis=1)
            mm = {k: (np.ascontiguousarray(v) if v.dtype == ml_dtypes.bfloat16
                      else np.ascontiguousarray(v, np.float32))
                  for k, v in m.items()}
            for k in ("xq", "xk", "xv", "wq", "wk", "wv"):
                mm[k] = mm[k].astype(ml_dtypes.bfloat16)
            mm["wo"] = np.ascontiguousarray(Wo[:, sl].T).astype(
                ml_dtypes.bfloat16)
            in_maps.append(mm)
    return in_maps


def kernel(query, key, value, Wq, bq, Wk, bk, Wv, bv, Wo, bo, proj,
           _trace=False):
    global _COMPILED
    from concourse import bass_utils
    args = [np.asarray(a, np.float32) for a in
            (query, key, value, Wq, bq, Wk, bk, Wv, bv, Wo, bo, proj)]
    if _COMPILED is None:
        _COMPILED = _build()
    in_maps = _prep_inputs(*args)
    res = bass_utils.run_bass_kernel_spmd(
        _COMPILED, in_maps, core_ids=list(range(8)), trace=_trace)
    out = np.empty((B, L, DIM), np.float32)
    bo_ = args[10]
    for b in range(B):
        out[b] = (res.results[2 * b]["outT"].T.astype(np.float32)
                  + res.results[2 * b + 1]["outT"].T.astype(np.float32)
                  + bo_)
    if _trace:
        kernel._last = res
    return out
